# revision 9
# baseline (speedup 1.0000x reference)
"""Masked-attention kernel for 8 TRN2 NeuronCores (batch-parallel sharding).

Per-core shard: 2 batches of [S=2048, D=128] Q/K/V + [S, S] bool mask.
Layout strategy (per core):
  - scores are computed TRANSPOSED (S^T[k, q]) so the PV matmul consumes the
    exp() output directly with V in its natural [k, d] layout.
  - the mask is folded into the scores inside the PE accumulation: an extra
    matmul per (k-tile, q-subtile) with the mask chunk (DMA-cast u8->fp8e4)
    as the stationary operand and a -448*I fp8 identity as the moving
    operand adds -448 to masked score entries; exp() then flushes them to 0.
  - softmax denominator: DVE accumulates exp tiles across k, a single
    ones-vector matmul reduces the partition axis, reciprocal on DVE, and
    the row is broadcast back over partitions via a DRAM-bounce DMA.
  - Q^T/K^T (and the final O^T -> O) transposes go through the DMA xbar
    (SBUF->SBUF, fp16), keeping the PE free for real matmuls.
"""

import numpy as np
import ml_dtypes

B, S, D = 16, 2048, 128
NCORES = 8
BP = B // NCORES  # batches per core
P = 128
QC = 1024  # q-chunk (columns of the transposed score tile)
NQC = S // QC
NKT = S // P  # k tiles
NQS = QC // P  # q subtiles per chunk
MM_N = 512  # matmul moving free dim
SCALE = 1.0 / float(np.sqrt(128.0))
MASK_NEG = -448.0

_CACHE = {}


def build_nc():
    import concourse.mybir as mybir
    import concourse.tile as tile
    from concourse import bacc
    from concourse.bass import AP

    fp16 = mybir.dt.float16
    fp32 = mybir.dt.float32
    Exp = mybir.ActivationFunctionType.Exp

    nc = bacc.Bacc("TRN2", target_bir_lowering=False, debug=False,
                   num_devices=NCORES)

    Qd = nc.dram_tensor("Q", [BP, S, D], fp32, kind="ExternalInput")
    Kd = nc.dram_tensor("K", [BP, S, D], fp32, kind="ExternalInput")
    Vd = nc.dram_tensor("V", [BP, S, D], fp32, kind="ExternalInput")
    Md = nc.dram_tensor("mask", [BP, S, S], mybir.dt.uint8, kind="ExternalInput")
    # run-count knob for differential HW timing (kernel() passes 1)
    Id = nc.dram_tensor("iters", [1, 1], mybir.dt.int32, kind="ExternalInput")
    Od = nc.dram_tensor("out", [BP, S, D], fp32, kind="ExternalOutput")
    scratch = nc.dram_tensor("den_scratch", [BP * NQC, QC], fp32)

    negI_np = (MASK_NEG * np.eye(P, dtype=np.float32)).astype(np.float16)
    negI_dram = nc.inline_tensor(negI_np, name="negI_const")

    with tile.TileContext(nc) as tc:
        with tc.tile_pool(name="consts", bufs=1) as consts, \
             tc.tile_pool(name="stag", bufs=2) as stag, \
             tc.tile_pool(name="qkv", bufs=2) as qkv, \
             tc.tile_pool(name="maskp", bufs=2) as maskp, \
             tc.tile_pool(name="pp", bufs=3) as pp, \
             tc.tile_pool(name="accp", bufs=2) as accp, \
             tc.tile_pool(name="outp", bufs=2) as outp, \
             tc.tile_pool(name="recp", bufs=2) as recp, \
             tc.tile_pool(name="spsum", bufs=2, space="PSUM") as spsum, \
             tc.tile_pool(name="opsum", bufs=2, space="PSUM") as opsum:

            negI = consts.tile([P, P], fp16)
            nc.sync.dma_start(out=negI[:, :], in_=negI_dram.ap())
            ones_col = consts.tile([P, 1], fp16)
            nc.vector.memset(ones_col, 1.0)
            it_sb = consts.tile([1, 1], mybir.dt.int32)
            nc.sync.dma_start(out=it_sb[:, :], in_=Id.ap())
            n_iters = nc.values_load(it_sb[:, :],
                                     skip_runtime_bounds_check=True)

            with tc.For_i(0, n_iters, 1):
                _kernel_body(nc, tc, mybir, Qd, Kd, Vd, Md, Od, scratch,
                             negI, ones_col, stag, qkv, maskp, pp, accp,
                             outp, recp, spsum, opsum)
    nc.compile()
    return nc


def _kernel_body(nc, tc, mybir, Qd, Kd, Vd, Md, Od, scratch, negI, ones_col,
                 stag, qkv, maskp, pp, accp, outp, recp, spsum, opsum):
    from concourse.bass import AP

    fp16 = mybir.dt.float16
    fp32 = mybir.dt.float32
    Exp = mybir.ActivationFunctionType.Exp

    if True:
            for b in range(BP):
                qh = stag.tile([P, NKT, D], fp16, name="qh")
                nc.gpsimd.dma_start(
                    out=qh[:, :, :],
                    in_=Qd.ap()[b].rearrange("(t p) d -> p t d", p=P))
                kh = stag.tile([P, NKT, D], fp16, name="kh")
                nc.gpsimd.dma_start(
                    out=kh[:, :, :],
                    in_=Kd.ap()[b].rearrange("(t p) d -> p t d", p=P))
                qt = qkv.tile([P, S], fp16, name="qt")
                ktt = qkv.tile([P, S], fp16, name="ktt")
                for t in range(NKT):
                    nc.sync.dma_start(out=qt[:, t * P:(t + 1) * P],
                                      in_=qh[:, t, :], transpose=True)
                    nc.sync.dma_start(out=ktt[:, t * P:(t + 1) * P],
                                      in_=kh[:, t, :], transpose=True)
                vsb = qkv.tile([P, NKT, D], fp16, name="vsb")
                nc.gpsimd.dma_start(
                    out=vsb[:, :, :],
                    in_=Vd.ap()[b].rearrange("(t p) d -> p t d", p=P))

                for qc in range(NQC):
                    mf = maskp.tile([P, NQS, S], fp16, name="mf")
                    nc.gpsimd.dma_start(
                        out=mf[:, :, :],
                        in_=Md.ap()[b, qc * QC:(qc + 1) * QC, :]
                            .rearrange("(s p) k -> p s k", p=P))
                    acc = accp.tile([P, QC], fp16, name="acc")
                    ops = opsum.tile([P, QC], fp32, name="opsum")
                    for kt in range(NKT):
                        sc = spsum.tile([P, QC], fp32, name="scores")
                        for n in range(0, QC, MM_N):
                            nc.tensor.matmul(
                                sc[:, n:n + MM_N],
                                lhsT=ktt[:, kt * P:(kt + 1) * P],
                                rhs=qt[:, qc * QC + n:qc * QC + n + MM_N],
                                start=True, stop=False, skip_group_check=True)
                        for sq in range(NQS):
                            nc.tensor.matmul(
                                sc[:, sq * P:(sq + 1) * P],
                                lhsT=mf[:, sq, kt * P:(kt + 1) * P],
                                rhs=negI[:, :],
                                start=False,
                                stop=(sq % (MM_N // P) == MM_N // P - 1),
                                skip_group_check=True)
                        pt = pp.tile([P, QC], fp16, name="pt")
                        nc.scalar.activation(out=pt[:, :], in_=sc[:, :],
                                             func=Exp, scale=SCALE)
                        if kt == 0:
                            nc.vector.tensor_copy(out=acc[:, :], in_=pt[:, :])
                        else:
                            nc.vector.tensor_add(out=acc[:, :], in0=acc[:, :],
                                                 in1=pt[:, :])
                        for n in range(0, QC, MM_N):
                            nc.tensor.matmul(
                                ops[:, n:n + MM_N],
                                lhsT=vsb[:, kt, :],
                                rhs=pt[:, n:n + MM_N],
                                start=(kt == 0), stop=(kt == NKT - 1),
                                skip_group_check=True)

                    den = spsum.tile([1, QC], fp32, name="scores")
                    for n in range(0, QC, MM_N):
                        nc.tensor.matmul(den[:, n:n + MM_N],
                                         lhsT=ones_col[:, :],
                                         rhs=acc[:, n:n + MM_N],
                                         start=True, stop=True,
                                         skip_group_check=True)
                    rec = recp.tile([1, QC], fp32, name="rec")
                    nc.vector.reciprocal(out=rec[:, :], in_=den[:, :])
                    sidx = b * NQC + qc
                    nc.gpsimd.dma_start(out=scratch.ap()[sidx, :],
                                        in_=rec[:, :])
                    recb = recp.tile([P, QC], fp32, name="recb")
                    row = scratch.ap()[sidx]
                    bcast = AP(tensor=row.tensor, offset=row.offset,
                               ap=[[0, P]] + list(row.ap))
                    nc.gpsimd.dma_start(out=recb[:, :], in_=bcast)

                    ot = outp.tile([P, QC], fp16, name="ot")
                    nc.vector.tensor_tensor(out=ot[:, :], in0=ops[:, :],
                                            in1=recb[:, :],
                                            op=mybir.AluOpType.mult)
                    osb = outp.tile([P, NQS, D], fp16, name="osb")
                    for t in range(NQS):
                        nc.sync.dma_start(out=osb[:, t, :],
                                          in_=ot[:, t * P:(t + 1) * P],
                                          transpose=True)
                    nc.gpsimd.dma_start(
                        out=Od.ap()[b, qc * QC:(qc + 1) * QC, :]
                            .rearrange("(t p) d -> p t d", p=P),
                        in_=osb[:, :, :])


def _get_nc():
    if "nc" not in _CACHE:
        _CACHE["nc"] = build_nc()
    return _CACHE["nc"]


def kernel(Q, K, V, mask, dk=128):
    from concourse.bass_utils import run_bass_kernel_spmd

    assert int(dk) == 128
    Q = np.ascontiguousarray(np.asarray(Q, dtype=np.float32))
    K = np.ascontiguousarray(np.asarray(K, dtype=np.float32))
    V = np.ascontiguousarray(np.asarray(V, dtype=np.float32))
    mask_u8 = np.ascontiguousarray(np.asarray(mask)).astype(np.uint8)

    nc = _get_nc()
    iters = np.ones((1, 1), dtype=np.int32)
    in_maps = []
    for c in range(NCORES):
        sl = slice(c * BP, (c + 1) * BP)
        in_maps.append({
            "Q": np.ascontiguousarray(Q[sl]),
            "K": np.ascontiguousarray(K[sl]),
            "V": np.ascontiguousarray(V[sl]),
            "mask": np.ascontiguousarray(mask_u8[sl]),
            "iters": iters,
        })
    res = run_bass_kernel_spmd(nc, in_maps, core_ids=list(range(NCORES)))
    return np.concatenate([r["out"] for r in res.results], axis=0)


# revision 12
# speedup vs baseline: 3.5848x; 3.5848x over previous
"""Masked-attention kernel for 8 TRN2 NeuronCores (batch-parallel sharding).

Per-core shard: 2 batches of [S=2048, D=128] Q/K/V + [S, S] bool mask.
Layout strategy (per core):
  - scores are computed TRANSPOSED (S^T[k, q]) so the PV matmul consumes the
    exp() output directly with V in its natural [k, d] layout.
  - the mask is folded into the scores inside the PE accumulation: an extra
    matmul per (k-tile, q-subtile) with the mask chunk (DMA-cast u8->fp8e4)
    as the stationary operand and a -240*I fp8 identity as the moving
    operand; exp() then flushes masked entries to ~0.
  - softmax denominator: DVE accumulates exp tiles across k-tiles, then per
    q-subtile one [acc-chunk]^T @ ones matmul gives the denominator as a
    PSUM column; reciprocal on DVE; applied as a per-partition scalar after
    the final transpose.
  - Q^T/K^T and O^T->O transposes use single batched DMA-xbar instructions
    (SBUF->SBUF, fp16, per-128-column block transposes).
"""

import numpy as np
import ml_dtypes

B, S, D = 16, 2048, 128
NCORES = 8
BP = B // NCORES  # batches per core
P = 128
QC = 1024  # q-chunk (columns of the transposed score tile)
NQC = S // QC
NKT = S // P  # k tiles
NQS = QC // P  # q subtiles per chunk
MM_N = 512  # matmul moving free dim
SCALE = 1.0 / float(np.sqrt(128.0))
MASK_NEG = -240.0

_CACHE = {}


def build_nc(loop=True):
    import concourse.mybir as mybir
    import concourse.tile as tile
    from concourse import bacc

    fp16 = mybir.dt.float16
    fp32 = mybir.dt.float32

    nc = bacc.Bacc("TRN2", target_bir_lowering=False, debug=False,
                   num_devices=NCORES)

    Qd = nc.dram_tensor("Q", [BP, S, D], fp32, kind="ExternalInput")
    Kd = nc.dram_tensor("K", [BP, S, D], fp32, kind="ExternalInput")
    Vd = nc.dram_tensor("V", [BP, S, D], fp32, kind="ExternalInput")
    Md = nc.dram_tensor("mask", [BP, S, S], mybir.dt.uint8, kind="ExternalInput")
    # run-count knob for differential HW timing (kernel() passes 1)
    Id = nc.dram_tensor("iters", [1, 1], mybir.dt.int32, kind="ExternalInput")
    Od = nc.dram_tensor("out", [BP, S, D], fp32, kind="ExternalOutput")

    negI_np = (MASK_NEG * np.eye(P, dtype=np.float32)).astype(
        ml_dtypes.float8_e4m3)
    negI_dram = nc.inline_tensor(negI_np, name="negI_const")

    with tile.TileContext(nc) as tc:
        with tc.tile_pool(name="consts", bufs=1) as consts, \
             tc.tile_pool(name="stag", bufs=2) as stag, \
             tc.tile_pool(name="qkv", bufs=1) as qkv, \
             tc.tile_pool(name="maskp", bufs=2) as maskp, \
             tc.tile_pool(name="pp", bufs=3) as pp, \
             tc.tile_pool(name="accp", bufs=2) as accp, \
             tc.tile_pool(name="outp", bufs=2) as outp, \
             tc.tile_pool(name="spsum", bufs=2, space="PSUM") as spsum, \
             tc.tile_pool(name="opsum", bufs=2, space="PSUM") as opsum:

            negI = consts.tile([P, P], mybir.dt.float8e4)
            nc.sync.dma_start(out=negI[:, :], in_=negI_dram.ap())
            ones_col = consts.tile([P, 1], fp16)
            nc.vector.memset(ones_col, 1.0)

            pools = (stag, qkv, maskp, pp, accp, outp, spsum, opsum)
            if loop:
                it_sb = consts.tile([1, 1], mybir.dt.int32)
                nc.sync.dma_start(out=it_sb[:, :], in_=Id.ap())
                n_iters = nc.values_load(it_sb[:, :],
                                         skip_runtime_bounds_check=True)
                with tc.For_i(0, n_iters, 1):
                    _kernel_body(nc, mybir, Qd, Kd, Vd, Md, Od, negI,
                                 ones_col, *pools)
            else:
                _kernel_body(nc, mybir, Qd, Kd, Vd, Md, Od, negI,
                             ones_col, *pools)
    nc.compile()
    return nc


def _kernel_body(nc, mybir, Qd, Kd, Vd, Md, Od, negI, ones_col,
                 stag, qkv, maskp, pp, accp, outp, spsum, opsum):
    fp16 = mybir.dt.float16
    fp32 = mybir.dt.float32
    fp8 = mybir.dt.float8e4
    Exp = mybir.ActivationFunctionType.Exp

    # ---- prep: load + cast + transpose Q/K, load V (both batches) ----
    qts, ktts, vsbs = [], [], []
    for b in range(BP):
        qh = stag.tile([P, S], fp16, name="qh")
        nc.gpsimd.dma_start(
            out=qh[:, :].rearrange("p (t d) -> p t d", t=NKT),
            in_=Qd.ap()[b].rearrange("(t p) d -> p t d", p=P))
        kh = stag.tile([P, S], fp16, name="kh")
        nc.gpsimd.dma_start(
            out=kh[:, :].rearrange("p (t d) -> p t d", t=NKT),
            in_=Kd.ap()[b].rearrange("(t p) d -> p t d", p=P))
        qt = qkv.tile([P, S], fp16, name=f"qt{b}")
        ktt = qkv.tile([P, S], fp16, name=f"ktt{b}")
        nc.sync.dma_start_transpose(
            out=qt[:, :].rearrange("d (t q) -> d t q", t=NKT), in_=qh[:, :])
        nc.sync.dma_start_transpose(
            out=ktt[:, :].rearrange("d (t k) -> d t k", t=NKT), in_=kh[:, :])
        vsb = qkv.tile([P, NKT, D], fp16, name=f"vsb{b}")
        nc.gpsimd.dma_start(
            out=vsb[:, :, :],
            in_=Vd.ap()[b].rearrange("(t p) d -> p t d", p=P))
        qts.append(qt)
        ktts.append(ktt)
        vsbs.append(vsb)

    # ---- main flash loop over (batch, q-chunk, k-tile) ----
    for b in range(BP):
        qt, ktt, vsb = qts[b], ktts[b], vsbs[b]
        for qc in range(NQC):
            mf = maskp.tile([P, NQS, S], fp8, name="mf")
            nc.gpsimd.dma_start(
                out=mf[:, :, :],
                in_=Md.ap()[b, qc * QC:(qc + 1) * QC, :]
                    .rearrange("(s p) k -> p s k", p=P))
            acc = accp.tile([P, QC], fp16, name="acc")
            ops = opsum.tile([P, QC], fp32, name="opsum")
            for kt in range(NKT):
                sc = spsum.tile([P, QC], fp32, name="scores")
                for n in range(0, QC, MM_N):
                    nc.tensor.matmul(
                        sc[:, n:n + MM_N],
                        lhsT=ktt[:, kt * P:(kt + 1) * P],
                        rhs=qt[:, qc * QC + n:qc * QC + n + MM_N],
                        start=True, stop=False, skip_group_check=True)
                for sq in range(NQS):
                    nc.tensor.matmul(
                        sc[:, sq * P:(sq + 1) * P],
                        lhsT=mf[:, sq, kt * P:(kt + 1) * P],
                        rhs=negI[:, :],
                        start=False,
                        stop=(sq % (MM_N // P) == MM_N // P - 1),
                        skip_group_check=True)
                pt = pp.tile([P, QC], fp16, name="pt")
                nc.scalar.activation(out=pt[:, :], in_=sc[:, :],
                                     func=Exp, scale=SCALE)
                if kt == 0:
                    nc.vector.tensor_copy(out=acc[:, :], in_=pt[:, :])
                else:
                    nc.vector.tensor_add(out=acc[:, :], in0=acc[:, :],
                                         in1=pt[:, :])
                for n in range(0, QC, MM_N):
                    nc.tensor.matmul(
                        ops[:, n:n + MM_N],
                        lhsT=vsb[:, kt, :],
                        rhs=pt[:, n:n + MM_N],
                        start=(kt == 0), stop=(kt == NKT - 1),
                        skip_group_check=True)

            # denominator as a PSUM column per q-subtile:
            # den[q_local, sq] = sum_k acc[k, sq*128 + q_local]
            den = spsum.tile([P, NQS], fp32, name="scores")
            for sq in range(NQS):
                nc.tensor.matmul(den[:, sq:sq + 1],
                                 lhsT=acc[:, sq * P:(sq + 1) * P],
                                 rhs=ones_col[:, :],
                                 start=True, stop=True,
                                 skip_group_check=True)
            rcol = outp.tile([P, NQS], fp32, name="rcol")
            nc.vector.reciprocal(out=rcol[:, :], in_=den[:, :])

            ot = outp.tile([P, QC], fp16, name="ot")
            nc.vector.tensor_copy(out=ot[:, :], in_=ops[:, :])
            osb = outp.tile([P, NQS, D], fp16, name="osb")
            nc.sync.dma_start_transpose(out=osb[:, :, :], in_=ot[:, :])
            osf = outp.tile([P, NQS, D], fp32, name="osf")
            for t in range(NQS):
                nc.vector.tensor_scalar_mul(out=osf[:, t, :],
                                            in0=osb[:, t, :],
                                            scalar1=rcol[:, t:t + 1])
            nc.sync.dma_start(
                out=Od.ap()[b, qc * QC:(qc + 1) * QC, :]
                    .rearrange("(t p) d -> p t d", p=P),
                in_=osf[:, :, :])


def _get_nc():
    if "nc" not in _CACHE:
        _CACHE["nc"] = build_nc()
    return _CACHE["nc"]


def kernel(Q, K, V, mask, dk=128):
    from concourse.bass_utils import run_bass_kernel_spmd

    assert int(dk) == 128
    Q = np.ascontiguousarray(np.asarray(Q, dtype=np.float32))
    K = np.ascontiguousarray(np.asarray(K, dtype=np.float32))
    V = np.ascontiguousarray(np.asarray(V, dtype=np.float32))
    mask_u8 = np.ascontiguousarray(np.asarray(mask)).astype(np.uint8)

    nc = _get_nc()
    iters = np.ones((1, 1), dtype=np.int32)
    in_maps = []
    for c in range(NCORES):
        sl = slice(c * BP, (c + 1) * BP)
        in_maps.append({
            "Q": np.ascontiguousarray(Q[sl]),
            "K": np.ascontiguousarray(K[sl]),
            "V": np.ascontiguousarray(V[sl]),
            "mask": np.ascontiguousarray(mask_u8[sl]),
            "iters": iters,
        })
    res = run_bass_kernel_spmd(nc, in_maps, core_ids=list(range(NCORES)))
    return np.concatenate([r["out"] for r in res.results], axis=0)
